# revision 1
# baseline (speedup 1.0000x reference)
"""Distributed Trainium2 (Bass) kernel for nn_Attention_53764400611491.

The reference module has HEADS == C == 64, so head_dim d = C//HEADS = 1.
With d = 1 the attention algebra collapses: per (batch b, head c)

    attn = q k^T            (outer product, [N,N])
    o    = attn @ v  =  q * (k . v)        <- a scalar per (b,c)!

so the whole module is

    out[b,c,n] = sum_c' wp[c,c'] * q[b,c',n] * s[b,c'] + x[b,c,n]
    q = wq @ x_b          s[b,c'] = sum_n (wk@x_b)[c',n] * (wv@x_b)[c',n]

and the [b,h,N,N] attention tensor never needs to exist.  Further, with
u = (wk+wv) @ x and d = (wk-wv) @ x:   s = (sum u^2 - sum d^2) / 4,
which lets the scalar (ACT) engine square straight out of PSUM (engines
may read at most one non-scalar PSUM operand per instruction).

Sharding over 8 NeuronCores: core i handles batch b = i//4 and output
n-chunk j = i%4 (256 of the 1024 flattened h*w positions).  Each core
receives the full x_b (rotated so its own chunk comes first), computes
s_b redundantly, and writes its 64x256 output chunk.  No collectives:
an 8-core AllReduce has a ~10us latency floor, far more than the ~1us
of redundant compute it would save.

Matmuls run as float32r (single-pass fp32, ~4x the fp32 rate; measured
end-to-end relative error ~4e-4).  x is DMA'd in 4 column-chunks over
three DMA rings; uv matmuls, ACT squares and DVE row-sum reduces
pipeline chunk by chunk; the final "+ x" is pre-accumulated into the
output PSUM bank by an identity matmul during PE idle time.
"""
import numpy as np

import concourse.bass as bass
import concourse.mybir as mybir
from concourse.bass_utils import run_bass_kernel_spmd

F32 = mybir.dt.float32
F32R = mybir.dt.float32r
MULT = mybir.AluOpType.mult
SUB = mybir.AluOpType.subtract
SQUARE = mybir.ActivationFunctionType.Square

B, C, H, W = 2, 64, 32, 32
N = H * W          # 1024
NCHUNK = N // 4    # 256 output columns per core


def _build_nc() -> bass.Bass:
    nc = bass.Bass()
    x_ext = nc.declare_dram_parameter("xr", [128, 512], F32R, isOutput=False)
    wkv_ext = nc.declare_dram_parameter("wkv", [128, 128], F32R, isOutput=False)
    wqp_ext = nc.declare_dram_parameter("wqp", [128, 128], F32R, isOutput=False)
    # out chunk [64,256] packed as [128,128]: partitions 0-63 = cols 0-127,
    # partitions 64-127 = cols 128-255 (full-width single DMA)
    o_ext = nc.declare_dram_parameter("out", [128, 128], F32, isOutput=True)

    from contextlib import ExitStack

    with ExitStack() as ctx:
        e = ctx.enter_context
        Wkv = e(nc.sbuf_tensor("Wkv", [128, 128], F32R))
        Wqp = e(nc.sbuf_tensor("Wqp", [128, 128], F32R))
        Xsb = e(nc.sbuf_tensor("Xsb", [128, 512], F32R))
        sq = e(nc.sbuf_tensor("sq", [128, 1024], F32))
        redc = e(nc.sbuf_tensor("redc", [128, 4], F32))
        redall = e(nc.sbuf_tensor("redall", [128, 1], F32))
        s4 = e(nc.sbuf_tensor("s4", [64, 1], F32))
        Qsb = e(nc.sbuf_tensor("Qsb", [64, 256], F32R))
        wpTs = e(nc.sbuf_tensor("wpTs", [64, 64], F32R))
        Fsb = e(nc.sbuf_tensor("Fsb", [64, 128], F32))
        Ftmp = e(nc.sbuf_tensor("Ftmp", [64, 128], F32))
        dummy = e(nc.sbuf_tensor("warmup", [1, 1], F32))
        uv1 = e(nc.psum_tensor("uv1", [128, 320], F32))
        uv2 = e(nc.psum_tensor("uv2", [128, 192], F32))
        uv3 = e(nc.psum_tensor("uv3", [128, 384], F32))
        uv4 = e(nc.psum_tensor("uv4", [128, 128], F32))
        Qp = e(nc.psum_tensor("Qp", [64, 256], F32))
        Op = e(nc.psum_tensor("Op", [64, 256], F32))
        wkv_sem = e(nc.semaphore("wkv_sem"))
        wqp_sem = e(nc.semaphore("wqp_sem"))
        xa1_sem = e(nc.semaphore("xa1_sem"))
        xa2_sem = e(nc.semaphore("xa2_sem"))
        xb1_sem = e(nc.semaphore("xb1_sem"))
        xb2_sem = e(nc.semaphore("xb2_sem"))
        pe_sem = e(nc.semaphore("pe_sem"))
        dv_sem = e(nc.semaphore("dv_sem"))
        act_sem = e(nc.semaphore("act_sem"))
        out_sem = e(nc.semaphore("out_sem"))
        block = e(nc.Block())

        def r(ap):
            return ap.bitcast(F32R)

        @block.sync
        def _(sync):
            sync.dma_start(Wkv[:], wkv_ext[:]).then_inc(wkv_sem, 16)
            sync.dma_start(Xsb[0:64, 320:512], x_ext[0:64, 320:512]).then_inc(xa2_sem, 16)
            sync.dma_start(Xsb[64:128, 384:512], x_ext[64:128, 384:512]).then_inc(xb2_sem, 16)
            sync.wait_ge(dv_sem, 8)
            sync.dma_start(o_ext[0:64, :], Fsb[:]).then_inc(out_sem, 16)
            sync.wait_ge(out_sem, 32)

        @block.gpsimd
        def _(gp):
            gp.dma_start(Xsb[64:128, 0:384], x_ext[64:128, 0:384]).then_inc(xb1_sem, 16)

        @block.tensor
        def _(pe):
            pe.wait_ge(wkv_sem, 16)
            pe.wait_ge(xa1_sem, 16)
            # u,d chunks: rows 0-63 = u = (wk+wv)x, rows 64-127 = d = (wk-wv)x
            pe.matmul(uv1[:], r(Wkv[0:64, :]), r(Xsb[0:64, 0:320]), start=True, stop=True).then_inc(pe_sem, 1)
            pe.wait_ge(xa2_sem, 16)
            pe.matmul(uv2[:], r(Wkv[0:64, :]), r(Xsb[0:64, 320:512]), start=True, stop=True).then_inc(pe_sem, 1)
            pe.wait_ge(xb1_sem, 16)
            pe.matmul(uv3[:], r(Wkv[64:128, :]), r(Xsb[64:128, 0:384]), start=True, stop=True).then_inc(pe_sem, 1)
            pe.wait_ge(xb2_sem, 16)
            pe.matmul(uv4[:], r(Wkv[64:128, :]), r(Xsb[64:128, 384:512]), start=True, stop=True).then_inc(pe_sem, 1)
            # q for own chunk
            pe.wait_ge(wqp_sem, 16)
            pe.matmul(Qp[:], r(Wqp[0:64, 0:64]), r(Xsb[0:64, 0:256]), start=True, stop=True).then_inc(pe_sem, 1)
            # preload x chunk into the output PSUM bank (identity matmul)
            pe.matmul(Op[:], r(Wqp[0:64, 64:128]), r(Xsb[0:64, 0:256]), start=True, stop=False).then_inc(pe_sem, 1)
            # out = (wp diag(s)) @ q + x  (accumulates into Op)
            pe.wait_ge(dv_sem, 7)
            pe.wait_ge(act_sem, 6)
            pe.matmul(Op[:], r(wpTs[:]), r(Qsb[:]), start=False, stop=True).then_inc(pe_sem, 1)

        @block.scalar
        def _(act):
            act.dma_start(Xsb[0:64, 0:320], x_ext[0:64, 0:320]).then_inc(xa1_sem, 16)
            act.dma_start(Wqp[:], wqp_ext[:]).then_inc(wqp_sem, 16)
            # warm the ACT table while DMAs are in flight
            act.activation(dummy[:], nc.const_aps.tensor(0.0, (1, 1), F32), SQUARE).then_inc(act_sem, 1)
            act.wait_ge(pe_sem, 1)
            act.activation(sq[:, 0:320], uv1[:], SQUARE).then_inc(act_sem, 1)
            act.wait_ge(pe_sem, 2)
            act.activation(sq[:, 320:512], uv2[:], SQUARE).then_inc(act_sem, 1)
            act.wait_ge(pe_sem, 3)
            act.activation(sq[:, 512:896], uv3[:], SQUARE).then_inc(act_sem, 1)
            act.wait_ge(pe_sem, 4)
            act.activation(sq[:, 896:1024], uv4[:], SQUARE).then_inc(act_sem, 1)
            # q copy PSUM->SBUF (with f32r rounding) off the DVE critical path
            act.wait_ge(pe_sem, 5)
            act.activation(Qsb[:], Qp[:], mybir.ActivationFunctionType.Copy).then_inc(act_sem, 1)
            # second half of the out chunk goes out on the ACT DMA ring
            act.wait_ge(dv_sem, 9)
            act.dma_start(o_ext[64:128, :], Ftmp[:]).then_inc(out_sem, 16)


        @block.vector
        def _(dv):
            # per-chunk row sums, each issued right behind its ACT square
            dv.wait_ge(act_sem, 2)
            dv.reduce_sum(redc[:, 0:1], sq[:, 0:320], axis=mybir.AxisListType.X).then_inc(dv_sem, 1)
            dv.wait_ge(act_sem, 3)
            dv.reduce_sum(redc[:, 1:2], sq[:, 320:512], axis=mybir.AxisListType.X).then_inc(dv_sem, 1)
            dv.wait_ge(act_sem, 4)
            dv.reduce_sum(redc[:, 2:3], sq[:, 512:896], axis=mybir.AxisListType.X).then_inc(dv_sem, 1)
            dv.wait_ge(act_sem, 5)
            dv.reduce_sum(redc[:, 3:4], sq[:, 896:1024], axis=mybir.AxisListType.X).then_inc(dv_sem, 1)
            dv.drain()  # redc landed (same-engine RAW, cheaper than sem wait)
            dv.reduce_sum(redall[:], redc[:], axis=mybir.AxisListType.X).then_inc(dv_sem, 1)
            dv.drain()  # redall landed
            # s4 = sum u^2 - sum d^2  (cross-base scalar operand)  = 4*s
            dv.tensor_scalar(s4[:], redall[0:64, :], redall[64:128, :], None, op0=SUB).then_inc(dv_sem, 1)
            dv.drain()  # s4 landed
            # wpTs = wp.T * s4 * 0.25  (fold the /4 of the +- identity)
            dv.tensor_scalar(wpTs[:], Wqp[64:128, 0:64], s4[:], 0.25, op0=MULT, op1=MULT).then_inc(dv_sem, 1)
            dv.wait_ge(pe_sem, 7)
            # out chunk halves PSUM -> SBUF
            dv.tensor_copy(Fsb[:], Op[:, 0:128]).then_inc(dv_sem, 1)
            dv.tensor_copy(Ftmp[:], Op[:, 128:256]).then_inc(dv_sem, 1)

    return nc


def _shard_inputs(x, wq, wk, wv, wp):
    """Full inputs -> list of 8 per-core {'xr','wkv','wqp'} dicts."""
    x = np.asarray(x, dtype=np.float32)
    wq, wk, wv, wp = (np.asarray(a, dtype=np.float32) for a in (wq, wk, wv, wp))
    xf = np.ascontiguousarray(x.reshape(B, C, N))
    kv = np.concatenate([(wk + wv).T, (wk - wv).T], axis=1)       # [64,128]
    wkv = np.ascontiguousarray(np.concatenate([kv, kv], axis=0))  # [128,128]
    eye = np.eye(64, dtype=np.float32)
    zero = np.zeros((64, 64), dtype=np.float32)
    wqp = np.ascontiguousarray(np.concatenate(
        [np.concatenate([wq.T, eye], axis=1),
         np.concatenate([wp.T, zero], axis=1)], axis=0))  # [128,128]
    in_maps = []
    for core in range(8):
        bb, j = core // 4, core % 4
        chunks = [xf[bb, :, ((j + t) % 4) * NCHUNK:(((j + t) % 4) + 1) * NCHUNK] for t in range(4)]
        upper = np.concatenate(chunks[0:2], axis=1)  # [64,512]
        lower = np.concatenate(chunks[2:4], axis=1)  # [64,512]
        xr = np.ascontiguousarray(np.concatenate([upper, lower], axis=0))  # [128,512]
        in_maps.append({"xr": xr, "wkv": wkv, "wqp": wqp})
    return in_maps


def _gather_outputs(results):
    """8 per-core {'out': [128,128]} -> full [b,C,h,w].

    Per-core out is the [64,256] chunk packed as [128,128]:
    partitions 0-63 = cols 0-127, partitions 64-127 = cols 128-255.
    """
    out = np.empty((B, C, N), dtype=np.float32)
    for core in range(8):
        bb, j = core // 4, core % 4
        o = np.asarray(results[core]["out"])
        chunk = np.concatenate([o[0:64, :], o[64:128, :]], axis=1)  # [64,256]
        out[bb, :, j * 256:(j + 1) * 256] = chunk
    return out.reshape(B, C, H, W)


_NC_CACHE = None


def kernel(x, wq, wk, wv, wp) -> np.ndarray:
    global _NC_CACHE
    if _NC_CACHE is None:
        _NC_CACHE = _build_nc()
    in_maps = _shard_inputs(x, wq, wk, wv, wp)
    last_err = None
    for _ in range(3):
        try:
            res = run_bass_kernel_spmd(_NC_CACHE, in_maps, core_ids=list(range(8)))
            return _gather_outputs(res.results)
        except Exception as exc:  # transient device-unrecoverable resets on retry
            last_err = exc
    raise last_err

